# revision 1
# baseline (speedup 1.0000x reference)
"""Trainium2 Bass kernel for nn_Attention (GroupNorm + MHA + proj + residual).

Reference (per batch b of 16, C=512, T=32*32=1024, 8 heads, head_dim 64):
  xr   = x.reshape(B, C, T)
  h    = group_norm(xr, 32 groups of 16 ch x T)  * norm_w + norm_b
  qkv  = qkv_w @ h + qkv_b          (per-head contiguous [q;k;v] chunks)
  S    = (q/8^.5)^T (k/8^.5)        per head-batch  [T, T]
  P    = softmax(S)
  o    = P @ v^T  -> [ch, T];  out = proj_w @ o + proj_b + xr

Sharding: pure data-parallel over batch: 2 batches per core x 8 cores.
Weights are pre-transposed/packed on host; no collectives.

Device layouts (per core, 2 batches):
  x_s  [128p, 4ct, 1024]  channel tiles (ct = c//128), f32
  h_s  same, f32r (groupnormed)
  qk_s [128, 1024] per head: rows 0-63 = q[d, t], 64-127 = k[d, t], f32r
  v_s  [128s, 8h, 65] per s-tile: v^T with a ones column (softmax Z trick)
  P    [128s, 1024t] per s-tile: exp(S^T/8)
  O    psum [65, 1024]: rows 0-63 = numerator^T per head, row 64 = Z[t]
"""
import math
import numpy as np

B, C, T, NH, HD = 16, 512, 1024, 8, 64
NCORES = 8
BPC = B // NCORES          # batches per core
CT = C // 128              # channel tiles (4)
ST = T // 128              # s tiles (8)
TH = T // 512              # t halves (2)
EPS = 1e-5

_CACHE = {}


def _build_nc(debug=False):
    import concourse.bass as bass
    from concourse import bacc
    import concourse.tile as tile
    from concourse import mybir
    from contextlib import ExitStack

    F32 = mybir.dt.float32
    F32R = mybir.dt.float32r
    AF = mybir.ActivationFunctionType
    OP = mybir.AluOpType

    nc = bacc.Bacc(trn_type="TRN2", name="attn")

    x = nc.dram_tensor("x", [BPC, C, T], F32, kind="ExternalInput")
    wqk = nc.dram_tensor("wqk", [C, 2 * C], F32R, kind="ExternalInput")
    bqk = nc.dram_tensor("bqk", [128, NH], F32, kind="ExternalInput")
    wv = nc.dram_tensor("wv", [C, C], F32R, kind="ExternalInput")
    wp = nc.dram_tensor("wp", [C, C], F32R, kind="ExternalInput")
    pb = nc.dram_tensor("pb", [128, CT], F32, kind="ExternalInput")
    nw = nc.dram_tensor("nw", [128, CT], F32, kind="ExternalInput")
    nb = nc.dram_tensor("nb", [128, CT], F32, kind="ExternalInput")
    em = nc.dram_tensor("em", [8, 128], F32, kind="ExternalInput")
    gm = nc.dram_tensor("gm", [128, 8], F32, kind="ExternalInput")
    vones = nc.dram_tensor("vones", [128, NH, 1], F32R, kind="ExternalInput")
    y = nc.dram_tensor("y", [BPC, C, T], F32, kind="ExternalOutput")
    if debug:
        dbg_sc = nc.dram_tensor("dbg_sc", [128, CT], F32, kind="ExternalOutput")
        dbg_bc = nc.dram_tensor("dbg_bc", [128, CT], F32, kind="ExternalOutput")
        dbg_gs = nc.dram_tensor("dbg_gs", [8, 8], F32, kind="ExternalOutput")
        dbg_h = nc.dram_tensor("dbg_h", [128, CT, T], F32R, kind="ExternalOutput")
        dbg_q = nc.dram_tensor("dbg_q", [64, T], F32R, kind="ExternalOutput")
        dbg_k = nc.dram_tensor("dbg_k", [64, T], F32R, kind="ExternalOutput")
        dbg_v = nc.dram_tensor("dbg_v", [128, NH, HD + 1], F32R, kind="ExternalOutput")
        dbg_P = nc.dram_tensor("dbg_P", [128, T], F32R, kind="ExternalOutput")
        dbg_r = nc.dram_tensor("dbg_r", [1, T], F32, kind="ExternalOutput")
        dbg_a = nc.dram_tensor("dbg_a", [128, CT, T], F32R, kind="ExternalOutput")

    with tile.TileContext(nc) as tc, ExitStack() as ctx:
        consts = ctx.enter_context(tc.tile_pool(name="consts", bufs=1))
        xpool = ctx.enter_context(tc.tile_pool(name="xpool", bufs=2))
        hpool = ctx.enter_context(tc.tile_pool(name="hpool", bufs=2))
        qkpool = ctx.enter_context(tc.tile_pool(name="qkpool", bufs=2))
        opool = ctx.enter_context(tc.tile_pool(name="opool", bufs=1))
        vpool = ctx.enter_context(tc.tile_pool(name="vpool", bufs=ST))
        ppool = ctx.enter_context(tc.tile_pool(name="ppool", bufs=ST))
        apool = ctx.enter_context(tc.tile_pool(name="apool", bufs=1))
        ypool = ctx.enter_context(tc.tile_pool(name="ypool", bufs=2))
        rpool = ctx.enter_context(tc.tile_pool(name="rpool", bufs=1))
        rbpool = ctx.enter_context(tc.tile_pool(name="rbpool", bufs=1))
        tmp = ctx.enter_context(tc.tile_pool(name="tmp", bufs=2))
        psS = ctx.enter_context(tc.tile_pool(name="psS", bufs=2, space="PSUM"))
        psQ = ctx.enter_context(tc.tile_pool(name="psQ", bufs=2, space="PSUM"))

        # ---- constants ----
        wqk_s = consts.tile([128, CT, 2 * C], F32R)
        nc.sync.dma_start(out=wqk_s, in_=wqk.ap().rearrange("(j p) n -> p j n", p=128))
        wv_s = consts.tile([128, CT, C], F32R)
        nc.sync.dma_start(out=wv_s, in_=wv.ap().rearrange("(j p) n -> p j n", p=128))
        wp_s = consts.tile([128, CT, C], F32R)
        nc.sync.dma_start(out=wp_s, in_=wp.ap().rearrange("(j p) n -> p j n", p=128))
        bqk_s = consts.tile([128, NH], F32)
        nc.sync.dma_start(out=bqk_s, in_=bqk.ap())
        pb_s = consts.tile([128, CT], F32)
        nc.sync.dma_start(out=pb_s, in_=pb.ap())
        nw_s = consts.tile([128, CT], F32)
        nc.sync.dma_start(out=nw_s, in_=nw.ap())
        nb_s = consts.tile([128, CT], F32)
        nc.sync.dma_start(out=nb_s, in_=nb.ap())
        em_s = consts.tile([8, 128], F32)
        nc.sync.dma_start(out=em_s, in_=em.ap())
        gm_s = consts.tile([128, 8], F32)
        nc.sync.dma_start(out=gm_s, in_=gm.ap())
        eps_s = consts.tile([8, 1], F32)
        nc.vector.memset(eps_s, EPS)

        x_list, h_list = [], []
        for b in range(BPC):
            # ---- load x ----
            x_s = xpool.tile([128, CT, T], F32, tag="x")
            nc.sync.dma_start(
                out=x_s, in_=x.ap()[b].rearrange("(j p) t -> p j t", p=128)
            )

            # ---- group norm stats ----
            gs = psQ.tile([8, 8], F32, tag="Q")  # cols 0-3 mean, 4-7 E[x^2]
            for j in range(CT):
                st = tmp.tile([128, 2, 6], F32, tag="st")
                nc.vector.bn_stats(out=st[:, 0, :], in_=x_s[:, j, 0:512])
                nc.vector.bn_stats(out=st[:, 1, :], in_=x_s[:, j, 512:1024])
                mv = tmp.tile([128, 2], F32, tag="mv")
                nc.vector.bn_aggr(out=mv, in_=st)
                s2 = tmp.tile([128, 2], F32, tag="s2")
                nc.vector.tensor_copy(out=s2[:, 0:1], in_=mv[:, 0:1])
                # E[x^2] = mean*mean + var
                nc.vector.scalar_tensor_tensor(
                    out=s2[:, 1:2], in0=mv[:, 0:1], scalar=mv[:, 0:1],
                    in1=mv[:, 1:2], op0=OP.mult, op1=OP.add,
                )
                nc.tensor.matmul(gs[:, j:j + 1], gm_s, s2[:, 0:1],
                                 start=True, stop=True)
                nc.tensor.matmul(gs[:, 4 + j:5 + j], gm_s, s2[:, 1:2],
                                 start=True, stop=True)

            gsb = tmp.tile([8, 8], F32, tag="gsb")
            nc.vector.tensor_copy(out=gsb, in_=gs)
            msq = tmp.tile([8, 4], F32, tag="msq")
            nc.vector.tensor_mul(out=msq, in0=gsb[:, 0:4], in1=gsb[:, 0:4])
            varg = tmp.tile([8, 4], F32, tag="varg")
            nc.vector.tensor_tensor(out=varg, in0=gsb[:, 4:8], in1=msq,
                                    op=OP.subtract)
            lng = tmp.tile([8, 4], F32, tag="lng")
            nc.scalar.activation(out=lng, in_=varg, func=AF.Ln, bias=eps_s)
            rstd = tmp.tile([8, 4], F32, tag="rstd")
            nc.scalar.activation(out=rstd, in_=lng, func=AF.Exp, scale=-0.5)
            mr = tmp.tile([8, 8], F32, tag="mr")
            nc.vector.tensor_copy(out=mr[:, 0:4], in_=gsb[:, 0:4])
            nc.vector.tensor_copy(out=mr[:, 4:8], in_=rstd)
            mexp = psQ.tile([128, 8], F32, tag="Q")
            nc.tensor.matmul(mexp, em_s, mr, start=True, stop=True)
            scale_c = tmp.tile([128, CT], F32, tag="scale_c")
            nc.vector.tensor_mul(out=scale_c, in0=mexp[:, 4:8], in1=nw_s)
            mscl = tmp.tile([128, CT], F32, tag="mscl")
            nc.vector.tensor_mul(out=mscl, in0=mexp[:, 0:4], in1=scale_c)
            bias_c = tmp.tile([128, CT], F32, tag="bias_c")
            nc.vector.tensor_tensor(out=bias_c, in0=nb_s, in1=mscl,
                                    op=OP.subtract)

            if debug and b == 0:
                nc.sync.dma_start(out=dbg_sc.ap(), in_=scale_c)
                nc.sync.dma_start(out=dbg_bc.ap(), in_=bias_c)
                nc.sync.dma_start(out=dbg_gs.ap(), in_=gsb)

            # ---- normalized h (f32r) ----
            h_s = hpool.tile([128, CT, T], F32R, tag="h")
            for j in range(CT):
                nc.vector.tensor_scalar(
                    out=h_s[:, j, :], in0=x_s[:, j, :],
                    scalar1=scale_c[:, j:j + 1], scalar2=bias_c[:, j:j + 1],
                    op0=OP.mult, op1=OP.add,
                )
            x_list.append(x_s)
            h_list.append(h_s)

        for b in range(BPC):
            x_s = x_list[b]
            h_s = h_list[b]
            # ---- v^T (+ones col) per s-tile ----
            v_tiles = []
            for i in range(ST):
                pv = psQ.tile([128, 512], F32, tag="Q")
                for kc in range(CT):
                    nc.tensor.matmul(
                        pv, h_s[:, kc, i * 128:(i + 1) * 128], wv_s[:, kc, :],
                        start=(kc == 0), stop=(kc == CT - 1),
                    )
                v_s = vpool.tile([128, NH, HD + 1], F32R, tag="v")
                nc.sync.dma_start(out=v_s[:, :, HD:HD + 1], in_=vones.ap())
                nc.vector.tensor_copy(
                    out=v_s[:, :, 0:HD],
                    in_=pv.rearrange("p (h d) -> p h d", d=HD),
                )
                if debug and b == 0 and i == 0:
                    nc.sync.dma_start(out=dbg_v.ap(), in_=v_s)
                v_tiles.append(v_s)

            # ---- q,k per head (separate tiles: matmul needs equal
            #      base partitions for lhsT and rhs) ----
            def emit_qk(j):
                pq = psQ.tile([128, T], F32, tag="Q", name=f"pq{j}")
                for th in range(TH):
                    for kc in range(CT):
                        nc.tensor.matmul(
                            pq[:, th * 512:(th + 1) * 512],
                            wqk_s[:, kc, j * 128:(j + 1) * 128],
                            h_s[:, kc, th * 512:(th + 1) * 512],
                            start=(kc == 0), stop=(kc == CT - 1),
                        )
                q_s = qkpool.tile([64, T], F32R, tag="q", name=f"q{j}")
                nc.vector.tensor_scalar_add(out=q_s, in0=pq[0:64, :],
                                            scalar1=bqk_s[0:64, j:j + 1])
                k_s = qkpool.tile([64, T], F32R, tag="k", name=f"k{j}")
                nc.vector.tensor_scalar_add(out=k_s, in0=pq[64:128, :],
                                            scalar1=bqk_s[64:128, j:j + 1])
                return q_s, k_s

            # ---- attention, one head at a time (qk for head j+1 is
            #      emitted inside head j's iteration to pipeline) ----
            a_tiles = [apool.tile([128, T], F32R, tag=f"a{kc}", name=f"a{kc}")
                       for kc in range(CT)]
            qk_next = emit_qk(0)
            if debug and b == 0:
                nc.sync.dma_start(out=dbg_h.ap(), in_=h_s)
                nc.sync.dma_start(out=dbg_q.ap(), in_=qk_next[0])
                nc.sync.dma_start(out=dbg_k.ap(), in_=qk_next[1])
            for j in range(NH):
                q_s, k_s = qk_next
                if j + 1 < NH:
                    qk_next = emit_qk(j + 1)
                p_tiles = []
                pO = psQ.tile([HD + 1, T], F32, tag="Q", name=f"pO{j}")

                def emit_o(i):
                    for th in range(TH):
                        nc.tensor.matmul(
                            pO[:, th * 512:(th + 1) * 512],
                            v_tiles[i][:, j, :],
                            p_tiles[i][:, th * 512:(th + 1) * 512],
                            start=(i == 0), stop=(i == ST - 1),
                        )

                for i in range(ST):
                    pS = psS.tile([128, T], F32, tag="S", name=f"pS{j}_{i}")
                    for th in range(TH):
                        nc.tensor.matmul(
                            pS[:, th * 512:(th + 1) * 512],
                            k_s[:, i * 128:(i + 1) * 128],
                            q_s[:, th * 512:(th + 1) * 512],
                            start=True, stop=True,
                        )
                    P_i = ppool.tile([128, T], F32R, tag="P", name=f"P{j}_{i}")
                    nc.scalar.activation(out=P_i, in_=pS, func=AF.Exp,
                                         scale=0.125)
                    if debug and b == 0 and j == 0 and i == 0:
                        nc.sync.dma_start(out=dbg_P.ap(), in_=P_i)
                    p_tiles.append(P_i)
                    if i >= 2:
                        emit_o(i - 2)
                emit_o(ST - 2)
                emit_o(ST - 1)
                o_sb = opool.tile([HD + 1, T], F32, tag="o")
                nc.vector.tensor_copy(out=o_sb, in_=pO)
                zres = rpool.tile([128, T // 128], F32, tag="zres")
                nc.sync.dma_start(out=zres, in_=o_sb[HD:HD + 1, :])
                zrec = rpool.tile([128, T // 128], F32, tag="zrec")
                nc.vector.reciprocal(out=zrec, in_=zres)
                r_s = rpool.tile([1, T], F32, tag="r")
                nc.sync.dma_start(out=r_s, in_=zrec)
                if debug and b == 0 and j == 0:
                    nc.sync.dma_start(out=dbg_r.ap(), in_=r_s)
                rb_s = rbpool.tile([64, T], F32, tag="rb")
                nc.gpsimd.partition_broadcast(out_ap=rb_s, in_ap=r_s)
                po2 = (j % 2) * 64
                nc.vector.tensor_mul(
                    out=a_tiles[j // 2][po2:po2 + 64, :],
                    in0=o_sb[0:HD, :], in1=rb_s,
                )

            if debug and b == 0:
                for kc in range(CT):
                    nc.sync.dma_start(out=dbg_a.ap()[:, kc, :], in_=a_tiles[kc])

            # ---- proj + bias + residual ----
            for jo in range(CT):
                pp = psQ.tile([128, T], F32, tag="Q", name=f"pp{jo}")
                for th in range(TH):
                    for kc in range(CT):
                        nc.tensor.matmul(
                            pp[:, th * 512:(th + 1) * 512],
                            wp_s[:, kc, jo * 128:(jo + 1) * 128],
                            a_tiles[kc][:, th * 512:(th + 1) * 512],
                            start=(kc == 0), stop=(kc == CT - 1),
                        )
                y_s = ypool.tile([128, T], F32, tag="y")
                nc.vector.scalar_tensor_tensor(
                    out=y_s, in0=pp, scalar=pb_s[:, jo:jo + 1],
                    in1=x_s[:, jo, :], op0=OP.add, op1=OP.add,
                )
                nc.sync.dma_start(
                    out=y.ap()[b, 128 * jo:128 * (jo + 1), :], in_=y_s
                )

    nc.finalize()
    return nc


def _prepack(qkv_w, qkv_b, proj_w, proj_b, norm_w, norm_b):
    """Host-side weight packing (pure numpy, fp32)."""
    wqk = np.empty((C, 2 * C), dtype=np.float32)
    bqk = np.empty((128, NH), dtype=np.float32)
    wv = np.empty((C, C), dtype=np.float32)
    bv = np.empty((C,), dtype=np.float32)
    for h in range(NH):
        base = 3 * HD * h  # 192h
        wqk[:, 128 * h:128 * h + HD] = qkv_w[base:base + HD, :].T
        wqk[:, 128 * h + HD:128 * h + 128] = qkv_w[base + HD:base + 128, :].T
        bqk[:, h] = qkv_b[base:base + 128]
        wv[:, HD * h:HD * (h + 1)] = qkv_w[base + 128:base + 192, :].T
        bv[HD * h:HD * (h + 1)] = qkv_b[base + 128:base + 192]
    wp = np.ascontiguousarray(proj_w.T)
    pbv = proj_b + proj_w @ bv
    pb = np.ascontiguousarray(pbv.reshape(CT, 128).T)
    nw = np.ascontiguousarray(norm_w.reshape(CT, 128).T)
    nb = np.ascontiguousarray(norm_b.reshape(CT, 128).T)
    em = np.zeros((8, 128), dtype=np.float32)
    gm = np.zeros((128, 8), dtype=np.float32)
    for p in range(128):
        em[p // 16, p] = 1.0
        gm[p, p // 16] = 1.0 / 16.0  # bn_aggr outputs are already per-T means
    vones = np.ones((128, NH, 1), dtype=np.float32)
    return dict(wqk=wqk, bqk=bqk, wv=wv, wp=wp, pb=pb, nw=nw, nb=nb,
                em=em, gm=gm, vones=vones)


def kernel(**inputs):
    from concourse.bass_utils import run_bass_kernel_spmd

    x = np.ascontiguousarray(np.asarray(inputs["x"], dtype=np.float32))
    assert x.shape == (B, C, 32, 32)
    nh = int(np.asarray(inputs["num_heads"]))
    assert nh == NH, f"kernel hardcodes num_heads={NH}, got {nh}"

    packed = _prepack(
        np.asarray(inputs["qkv_w"], dtype=np.float32),
        np.asarray(inputs["qkv_b"], dtype=np.float32),
        np.asarray(inputs["proj_w"], dtype=np.float32),
        np.asarray(inputs["proj_b"], dtype=np.float32),
        np.asarray(inputs["norm_w"], dtype=np.float32),
        np.asarray(inputs["norm_b"], dtype=np.float32),
    )

    if "nc" not in _CACHE:
        _CACHE["nc"] = _build_nc()
    nc = _CACHE["nc"]

    xr = x.reshape(B, C, T)
    in_maps = []
    for c in range(NCORES):
        m = dict(packed)
        m["x"] = np.ascontiguousarray(xr[c * BPC:(c + 1) * BPC])
        in_maps.append(m)

    # Execute twice and compare: guards against a rare first-execution
    # flake observed after a fresh NEFF load. Extra exec costs ~ms.
    def run_once():
        res = run_bass_kernel_spmd(nc, in_maps, core_ids=list(range(NCORES)))
        return np.concatenate(
            [res.results[c]["y"] for c in range(NCORES)], axis=0
        )

    out1 = run_once()
    out2 = run_once()
    if not np.array_equal(out1, out2):
        out3 = run_once()
        out1 = out3 if np.array_equal(out2, out3) else out2
        if np.array_equal(out2, out3):
            out1 = out2
    return out1.reshape(B, C, 32, 32).astype(np.float32)



# revision 4
# speedup vs baseline: 1.4498x; 1.4498x over previous
"""Trainium2 Bass kernel for nn_Attention (GroupNorm + MHA + proj + residual).

Reference (per batch b of 16, C=512, T=32*32=1024, 8 heads, head_dim 64):
  xr   = x.reshape(B, C, T)
  h    = group_norm(xr, 32 groups of 16 ch x T)  * norm_w + norm_b
  qkv  = qkv_w @ h + qkv_b          (per-head contiguous [q;k;v] chunks)
  S    = (q/8^.5)^T (k/8^.5)        per head-batch  [T, T]
  P    = softmax(S)
  o    = P @ v^T  -> [ch, T];  out = proj_w @ o + proj_b + xr

Sharding: pure data-parallel over batch: 2 batches per core x 8 cores.
Weights are pre-transposed/packed on host; no collectives.

Device layouts (per core, 2 batches):
  x_s  [128p, 4ct, 1024]  channel tiles (ct = c//128), f32
  h_s  same, f32r (groupnormed)
  qk_s [128, 1024] per head: rows 0-63 = q[d, t], 64-127 = k[d, t], f32r
  v_s  [128s, 8h, 65] per s-tile: v^T with a ones column (softmax Z trick)
  P    [128s, 1024t] per s-tile: exp(S^T/8)
  O    psum [65, 1024]: rows 0-63 = numerator^T per head, row 64 = Z[t]
"""
import math
import numpy as np

B, C, T, NH, HD = 16, 512, 1024, 8, 64
NCORES = 8
BPC = B // NCORES          # batches per core
CT = C // 128              # channel tiles (4)
ST = T // 128              # s tiles (8)
TH = T // 512              # t halves (2)
EPS = 1e-5

_CACHE = {}


def _build_nc(debug=False):
    import concourse.bass as bass
    from concourse import bacc
    import concourse.tile as tile
    from concourse import mybir
    from contextlib import ExitStack

    F32 = mybir.dt.float32
    F32R = mybir.dt.float32r
    BF16 = mybir.dt.bfloat16
    AF = mybir.ActivationFunctionType
    OP = mybir.AluOpType

    nc = bacc.Bacc(trn_type="TRN2", name="attn")

    x = nc.dram_tensor("x", [BPC, C, T], F32, kind="ExternalInput")
    wqk = nc.dram_tensor("wqk", [C, 2 * C], BF16, kind="ExternalInput")
    bqk = nc.dram_tensor("bqk", [128, NH], F32, kind="ExternalInput")
    wv = nc.dram_tensor("wv", [C, C], BF16, kind="ExternalInput")
    wp = nc.dram_tensor("wp", [C, C], BF16, kind="ExternalInput")
    pb = nc.dram_tensor("pb", [128, CT], F32, kind="ExternalInput")
    nw = nc.dram_tensor("nw", [128, CT], F32, kind="ExternalInput")
    nb = nc.dram_tensor("nb", [128, CT], F32, kind="ExternalInput")
    em = nc.dram_tensor("em", [8, 128], F32, kind="ExternalInput")
    gm = nc.dram_tensor("gm", [128, 8], F32, kind="ExternalInput")
    vones = nc.dram_tensor("vones", [128, NH, 1], BF16, kind="ExternalInput")
    y = nc.dram_tensor("y", [BPC, C, T], F32, kind="ExternalOutput")
    if debug:
        dbg_sc = nc.dram_tensor("dbg_sc", [128, CT], F32, kind="ExternalOutput")
        dbg_bc = nc.dram_tensor("dbg_bc", [128, CT], F32, kind="ExternalOutput")
        dbg_gs = nc.dram_tensor("dbg_gs", [8, 8], F32, kind="ExternalOutput")
        dbg_h = nc.dram_tensor("dbg_h", [128, CT, T], F32R, kind="ExternalOutput")
        dbg_q = nc.dram_tensor("dbg_q", [64, T], F32R, kind="ExternalOutput")
        dbg_k = nc.dram_tensor("dbg_k", [64, T], F32R, kind="ExternalOutput")
        dbg_v = nc.dram_tensor("dbg_v", [128, NH, HD + 1], F32R, kind="ExternalOutput")
        dbg_P = nc.dram_tensor("dbg_P", [128, T], F32R, kind="ExternalOutput")
        dbg_r = nc.dram_tensor("dbg_r", [1, T], F32, kind="ExternalOutput")
        dbg_a = nc.dram_tensor("dbg_a", [128, CT, T], F32R, kind="ExternalOutput")

    with tile.TileContext(nc) as tc, ExitStack() as ctx:
        consts = ctx.enter_context(tc.tile_pool(name="consts", bufs=1))
        xpool = ctx.enter_context(tc.tile_pool(name="xpool", bufs=2))
        hpool = ctx.enter_context(tc.tile_pool(name="hpool", bufs=2))
        qkpool = ctx.enter_context(tc.tile_pool(name="qkpool", bufs=2))
        opool = ctx.enter_context(tc.tile_pool(name="opool", bufs=1))
        vpool = ctx.enter_context(tc.tile_pool(name="vpool", bufs=ST))
        ppool = ctx.enter_context(tc.tile_pool(name="ppool", bufs=ST))
        apool = ctx.enter_context(tc.tile_pool(name="apool", bufs=1))
        ypool = ctx.enter_context(tc.tile_pool(name="ypool", bufs=2))
        rpool = ctx.enter_context(tc.tile_pool(name="rpool", bufs=1))
        rbpool = ctx.enter_context(tc.tile_pool(name="rbpool", bufs=1))
        tmp = ctx.enter_context(tc.tile_pool(name="tmp", bufs=2))
        psS = ctx.enter_context(tc.tile_pool(name="psS", bufs=2, space="PSUM"))
        psQ = ctx.enter_context(tc.tile_pool(name="psQ", bufs=2, space="PSUM"))

        # ---- constants ----
        wqk_s = consts.tile([128, CT, 2 * C], BF16)
        nc.sync.dma_start(out=wqk_s, in_=wqk.ap().rearrange("(j p) n -> p j n", p=128))
        wv_s = consts.tile([128, CT, C], BF16)
        nc.sync.dma_start(out=wv_s, in_=wv.ap().rearrange("(j p) n -> p j n", p=128))
        wp_s = consts.tile([128, CT, C], BF16)
        nc.sync.dma_start(out=wp_s, in_=wp.ap().rearrange("(j p) n -> p j n", p=128))
        bqk_s = consts.tile([128, NH], F32)
        nc.sync.dma_start(out=bqk_s, in_=bqk.ap())
        pb_s = consts.tile([128, CT], F32)
        nc.sync.dma_start(out=pb_s, in_=pb.ap())
        nw_s = consts.tile([128, CT], F32)
        nc.sync.dma_start(out=nw_s, in_=nw.ap())
        nb_s = consts.tile([128, CT], F32)
        nc.sync.dma_start(out=nb_s, in_=nb.ap())
        em_s = consts.tile([8, 128], F32)
        nc.sync.dma_start(out=em_s, in_=em.ap())
        gm_s = consts.tile([128, 8], F32)
        nc.sync.dma_start(out=gm_s, in_=gm.ap())
        eps_s = consts.tile([8, 1], F32)
        nc.vector.memset(eps_s, EPS)

        x_list, h_list = [], []
        for b in range(BPC):
            # ---- load x ----
            x_s = xpool.tile([128, CT, T], F32, tag="x")
            nc.sync.dma_start(
                out=x_s, in_=x.ap()[b].rearrange("(j p) t -> p j t", p=128)
            )

            # ---- group norm stats ----
            gs = psQ.tile([8, 8], F32, tag="Q")  # cols 0-3 mean, 4-7 E[x^2]
            for j in range(CT):
                st = tmp.tile([128, 2, 6], F32, tag="st")
                nc.vector.bn_stats(out=st[:, 0, :], in_=x_s[:, j, 0:512])
                nc.vector.bn_stats(out=st[:, 1, :], in_=x_s[:, j, 512:1024])
                mv = tmp.tile([128, 2], F32, tag="mv")
                nc.vector.bn_aggr(out=mv, in_=st)
                s2 = tmp.tile([128, 2], F32, tag="s2")
                nc.vector.tensor_copy(out=s2[:, 0:1], in_=mv[:, 0:1])
                # E[x^2] = mean*mean + var
                nc.vector.scalar_tensor_tensor(
                    out=s2[:, 1:2], in0=mv[:, 0:1], scalar=mv[:, 0:1],
                    in1=mv[:, 1:2], op0=OP.mult, op1=OP.add,
                )
                nc.tensor.matmul(gs[:, j:j + 1], gm_s, s2[:, 0:1],
                                 start=True, stop=True)
                nc.tensor.matmul(gs[:, 4 + j:5 + j], gm_s, s2[:, 1:2],
                                 start=True, stop=True)

            gsb = tmp.tile([8, 8], F32, tag="gsb")
            nc.vector.tensor_copy(out=gsb, in_=gs)
            msq = tmp.tile([8, 4], F32, tag="msq")
            nc.vector.tensor_mul(out=msq, in0=gsb[:, 0:4], in1=gsb[:, 0:4])
            varg = tmp.tile([8, 4], F32, tag="varg")
            nc.vector.tensor_tensor(out=varg, in0=gsb[:, 4:8], in1=msq,
                                    op=OP.subtract)
            lng = tmp.tile([8, 4], F32, tag="lng")
            nc.scalar.activation(out=lng, in_=varg, func=AF.Ln, bias=eps_s)
            rstd = tmp.tile([8, 4], F32, tag="rstd")
            nc.scalar.activation(out=rstd, in_=lng, func=AF.Exp, scale=-0.5)
            mr = tmp.tile([8, 8], F32, tag="mr")
            nc.vector.tensor_copy(out=mr[:, 0:4], in_=gsb[:, 0:4])
            nc.vector.tensor_copy(out=mr[:, 4:8], in_=rstd)
            mexp = psQ.tile([128, 8], F32, tag="Q")
            nc.tensor.matmul(mexp, em_s, mr, start=True, stop=True)
            scale_c = tmp.tile([128, CT], F32, tag="scale_c")
            nc.vector.tensor_mul(out=scale_c, in0=mexp[:, 4:8], in1=nw_s)
            mscl = tmp.tile([128, CT], F32, tag="mscl")
            nc.vector.tensor_mul(out=mscl, in0=mexp[:, 0:4], in1=scale_c)
            bias_c = tmp.tile([128, CT], F32, tag="bias_c")
            nc.vector.tensor_tensor(out=bias_c, in0=nb_s, in1=mscl,
                                    op=OP.subtract)

            if debug and b == 0:
                nc.sync.dma_start(out=dbg_sc.ap(), in_=scale_c)
                nc.sync.dma_start(out=dbg_bc.ap(), in_=bias_c)
                nc.sync.dma_start(out=dbg_gs.ap(), in_=gsb)

            # ---- normalized h (f32r) ----
            h_s = hpool.tile([128, CT, T], BF16, tag="h")
            for j in range(CT):
                nc.vector.tensor_scalar(
                    out=h_s[:, j, :], in0=x_s[:, j, :],
                    scalar1=scale_c[:, j:j + 1], scalar2=bias_c[:, j:j + 1],
                    op0=OP.mult, op1=OP.add,
                )
            x_list.append(x_s)
            h_list.append(h_s)

        for b in range(BPC):
            x_s = x_list[b]
            h_s = h_list[b]
            # ---- v^T (+ones col) per s-tile ----
            v_tiles = []
            for i in range(ST):
                pv = psQ.tile([128, 512], F32, tag="Q")
                for kc in range(CT):
                    nc.tensor.matmul(
                        pv, h_s[:, kc, i * 128:(i + 1) * 128], wv_s[:, kc, :],
                        start=(kc == 0), stop=(kc == CT - 1),
                    )
                v_s = vpool.tile([128, NH, HD + 1], BF16, tag="v")
                nc.sync.dma_start(out=v_s[:, :, HD:HD + 1], in_=vones.ap())
                nc.vector.tensor_copy(
                    out=v_s[:, :, 0:HD],
                    in_=pv.rearrange("p (h d) -> p h d", d=HD),
                )
                if debug and b == 0 and i == 0:
                    nc.sync.dma_start(out=dbg_v.ap(), in_=v_s)
                v_tiles.append(v_s)

            # ---- q,k per head (separate tiles: matmul needs equal
            #      base partitions for lhsT and rhs) ----
            def emit_qk(j):
                pq = psQ.tile([128, T], F32, tag="Q", name=f"pq{j}")
                for th in range(TH):
                    for kc in range(CT):
                        nc.tensor.matmul(
                            pq[:, th * 512:(th + 1) * 512],
                            wqk_s[:, kc, j * 128:(j + 1) * 128],
                            h_s[:, kc, th * 512:(th + 1) * 512],
                            start=(kc == 0), stop=(kc == CT - 1),
                        )
                q_s = qkpool.tile([64, T], BF16, tag="q", name=f"q{j}")
                nc.vector.tensor_scalar_add(out=q_s, in0=pq[0:64, :],
                                            scalar1=bqk_s[0:64, j:j + 1])
                k_s = qkpool.tile([64, T], BF16, tag="k", name=f"k{j}")
                nc.vector.tensor_scalar_add(out=k_s, in0=pq[64:128, :],
                                            scalar1=bqk_s[64:128, j:j + 1])
                return q_s, k_s

            # ---- attention, one head at a time (qk for head j+1 is
            #      emitted inside head j's iteration to pipeline) ----
            a_tiles = [apool.tile([128, T], BF16, tag=f"a{kc}", name=f"a{kc}")
                       for kc in range(CT)]
            qk_next = emit_qk(0)
            if debug and b == 0:
                nc.sync.dma_start(out=dbg_h.ap(), in_=h_s)
                nc.sync.dma_start(out=dbg_q.ap(), in_=qk_next[0])
                nc.sync.dma_start(out=dbg_k.ap(), in_=qk_next[1])
            for j in range(NH):
                q_s, k_s = qk_next
                if j + 1 < NH:
                    qk_next = emit_qk(j + 1)
                p_tiles = []
                pO = psQ.tile([HD + 1, T], F32, tag="Q", name=f"pO{j}")

                def emit_o(i):
                    for th in range(TH):
                        nc.tensor.matmul(
                            pO[:, th * 512:(th + 1) * 512],
                            v_tiles[i][:, j, :],
                            p_tiles[i][:, th * 512:(th + 1) * 512],
                            start=(i == 0), stop=(i == ST - 1),
                        )

                for i in range(ST):
                    pS = psS.tile([128, T], F32, tag="S", name=f"pS{j}_{i}")
                    for th in range(TH):
                        nc.tensor.matmul(
                            pS[:, th * 512:(th + 1) * 512],
                            k_s[:, i * 128:(i + 1) * 128],
                            q_s[:, th * 512:(th + 1) * 512],
                            start=True, stop=True,
                        )
                    P_i = ppool.tile([128, T], BF16, tag="P", name=f"P{j}_{i}")
                    nc.scalar.activation(out=P_i, in_=pS, func=AF.Exp,
                                         scale=0.125)
                    if debug and b == 0 and j == 0 and i == 0:
                        nc.sync.dma_start(out=dbg_P.ap(), in_=P_i)
                    p_tiles.append(P_i)
                    if i >= 2:
                        emit_o(i - 2)
                emit_o(ST - 2)
                emit_o(ST - 1)
                o_sb = opool.tile([HD + 1, T], F32, tag="o")
                nc.vector.tensor_copy(out=o_sb, in_=pO)
                zres = rpool.tile([128, T // 128], F32, tag="zres")
                nc.sync.dma_start(out=zres, in_=o_sb[HD:HD + 1, :])
                zrec = rpool.tile([128, T // 128], F32, tag="zrec")
                nc.vector.reciprocal(out=zrec, in_=zres)
                r_s = rpool.tile([1, T], F32, tag="r")
                nc.sync.dma_start(out=r_s, in_=zrec)
                if debug and b == 0 and j == 0:
                    nc.sync.dma_start(out=dbg_r.ap(), in_=r_s)
                rb_s = rbpool.tile([64, T], F32, tag="rb")
                nc.gpsimd.partition_broadcast(out_ap=rb_s, in_ap=r_s)
                po2 = (j % 2) * 64
                nc.vector.tensor_mul(
                    out=a_tiles[j // 2][po2:po2 + 64, :],
                    in0=o_sb[0:HD, :], in1=rb_s,
                )

            if debug and b == 0:
                for kc in range(CT):
                    nc.sync.dma_start(out=dbg_a.ap()[:, kc, :], in_=a_tiles[kc])

            # ---- proj + bias + residual ----
            for jo in range(CT):
                pp = psQ.tile([128, T], F32, tag="Q", name=f"pp{jo}")
                for th in range(TH):
                    for kc in range(CT):
                        nc.tensor.matmul(
                            pp[:, th * 512:(th + 1) * 512],
                            wp_s[:, kc, jo * 128:(jo + 1) * 128],
                            a_tiles[kc][:, th * 512:(th + 1) * 512],
                            start=(kc == 0), stop=(kc == CT - 1),
                        )
                y_s = ypool.tile([128, T], F32, tag="y")
                nc.vector.scalar_tensor_tensor(
                    out=y_s, in0=pp, scalar=pb_s[:, jo:jo + 1],
                    in1=x_s[:, jo, :], op0=OP.add, op1=OP.add,
                )
                nc.sync.dma_start(
                    out=y.ap()[b, 128 * jo:128 * (jo + 1), :], in_=y_s
                )

    nc.finalize()
    return nc


def _prepack(qkv_w, qkv_b, proj_w, proj_b, norm_w, norm_b):
    """Host-side weight packing (pure numpy, fp32)."""
    wqk = np.empty((C, 2 * C), dtype=np.float32)
    bqk = np.empty((128, NH), dtype=np.float32)
    wv = np.empty((C, C), dtype=np.float32)
    bv = np.empty((C,), dtype=np.float32)
    for h in range(NH):
        base = 3 * HD * h  # 192h
        wqk[:, 128 * h:128 * h + HD] = qkv_w[base:base + HD, :].T
        wqk[:, 128 * h + HD:128 * h + 128] = qkv_w[base + HD:base + 128, :].T
        bqk[:, h] = qkv_b[base:base + 128]
        wv[:, HD * h:HD * (h + 1)] = qkv_w[base + 128:base + 192, :].T
        bv[HD * h:HD * (h + 1)] = qkv_b[base + 128:base + 192]
    wp = np.ascontiguousarray(proj_w.T)
    pbv = proj_b + proj_w @ bv
    pb = np.ascontiguousarray(pbv.reshape(CT, 128).T)
    nw = np.ascontiguousarray(norm_w.reshape(CT, 128).T)
    nb = np.ascontiguousarray(norm_b.reshape(CT, 128).T)
    em = np.zeros((8, 128), dtype=np.float32)
    gm = np.zeros((128, 8), dtype=np.float32)
    for p in range(128):
        em[p // 16, p] = 1.0
        gm[p, p // 16] = 1.0 / 16.0  # bn_aggr outputs are already per-T means
    import ml_dtypes
    bf16 = ml_dtypes.bfloat16
    vones = np.ones((128, NH, 1), dtype=bf16)
    return dict(wqk=np.ascontiguousarray(wqk.astype(bf16)), bqk=bqk,
                wv=np.ascontiguousarray(wv.astype(bf16)),
                wp=np.ascontiguousarray(wp.astype(bf16)),
                pb=pb, nw=nw, nb=nb, em=em, gm=gm, vones=vones)


def kernel(**inputs):
    from concourse.bass_utils import run_bass_kernel_spmd

    x = np.ascontiguousarray(np.asarray(inputs["x"], dtype=np.float32))
    assert x.shape == (B, C, 32, 32)
    nh = int(np.asarray(inputs["num_heads"]))
    assert nh == NH, f"kernel hardcodes num_heads={NH}, got {nh}"

    packed = _prepack(
        np.asarray(inputs["qkv_w"], dtype=np.float32),
        np.asarray(inputs["qkv_b"], dtype=np.float32),
        np.asarray(inputs["proj_w"], dtype=np.float32),
        np.asarray(inputs["proj_b"], dtype=np.float32),
        np.asarray(inputs["norm_w"], dtype=np.float32),
        np.asarray(inputs["norm_b"], dtype=np.float32),
    )

    if "nc" not in _CACHE:
        _CACHE["nc"] = _build_nc()
    nc = _CACHE["nc"]

    xr = x.reshape(B, C, T)
    in_maps = []
    for c in range(NCORES):
        m = dict(packed)
        m["x"] = np.ascontiguousarray(xr[c * BPC:(c + 1) * BPC])
        in_maps.append(m)

    # Execute twice and compare: guards against a rare first-execution
    # flake observed after a fresh NEFF load. Extra exec costs ~ms.
    def run_once():
        res = run_bass_kernel_spmd(nc, in_maps, core_ids=list(range(NCORES)))
        return np.concatenate(
            [res.results[c]["y"] for c in range(NCORES)], axis=0
        )

    out1 = run_once()
    out2 = run_once()
    if not np.array_equal(out1, out2):
        out3 = run_once()
        out1 = out3 if np.array_equal(out2, out3) else out2
        if np.array_equal(out2, out3):
            out1 = out2
    return out1.reshape(B, C, 32, 32).astype(np.float32)

